# revision 1
# baseline (speedup 1.0000x reference)
"""Depthwise 4x4 FIR blur (upfirdn2d-style) on 8 Trainium2 NeuronCores.

Input  x: (16, 512, 64, 64) f32, kernel: (4, 4) f32 (normalized binomial).
Output y: same shape as x, y[g] = conv2d(zero-pad(x[g], (2,1)x(2,1)), flip(kernel)).

Equivalent per-image formula (derived from the reference):
    y[i, j] = sum_{a,b in [0,4)} kernel[a, b] * x[i+1-a, j+1-b]   (zero outside)

Strategy (per core, 1024 images = 16 strips of 64):
  - Host prepads each strip into [128, 2116]: partition k in [0,64) = row k of
    the even image of a pair, k in [64,128) = row k-64 of the odd image; along
    the free dim 32 image pairs at stride 66 (64 data cols + 2 zero cols) plus
    4 lead zeros. Horizontal taps then become free-dim shifts whose
    out-of-image reads land on zeros; strips load as one dense ~1MB DMA and
    all 16 loads prefetch with no dependencies.
  - The whole 2D conv runs on the TensorEngine: for each horizontal tap b, a
    banded 128x128 matrix (vertical taps folded in, block-diagonal per image)
    multiplies a shifted slice of the strip; 4 float32r matmuls accumulate
    per PSUM bank (1 cycle/row at N>=256).
  - ScalarE evacuates PSUM -> packed SBUF out tile and issues the store of
    the permuted dense [128, 2048] strip; the host inverse-permutes. HW
    moves only dense, large-descriptor DMAs in both directions.
  Measured: ~126 us/core (8 cores in parallel, ~34.4 MB of HBM traffic per
  core; ~96 us HBM roofline + ~9 us NEFF startup), rel err 1.4e-4 vs the
  fp32 reference (float32r matmul precision).
"""

import numpy as np

import concourse.bass as bass
import concourse.tile as tile
from concourse import mybir
from concourse.bass_utils import run_bass_kernel_spmd

# The kernel-tail drain waits on every semaphore family the kernel touched
# (PE + ACT + up to 8 DMA lanes); walrus rejects instructions with that many
# sync waits. Split the drain into several drain instructions, each carrying
# at most 3 waits — semantically identical (SP executes them in sequence).
import bass_rust as _bass_rust
from concourse.tile_scheduler import N_PROCS as _N_PROCS

def _split_drain_and_barrier(self, tick_clock, wait_clock):
    ScopedClock = _bass_rust.ScopedClock
    VectorClock = _bass_rust.VectorClock
    gc = tick_clock.global_clock
    vals = [gc[p] for p in range(_N_PROCS)]
    nonzero = [p for p in range(_N_PROCS) if vals[p] > 0]
    for p in nonzero:
        pv = [vals[q] if q == p else 0 for q in range(_N_PROCS)]
        d = self.nc.sync.drain()
        wait_clock.add_sem_waits(d.ins, ScopedClock({None: VectorClock(pv)}))
    self.nc.sync.drain()

    self.nc.all_engine_barrier()
    assert self.sems is not None
    popped = self.nc._tile_sem_poison_stack.pop()
    assert popped is self._sem_poison
    self.nc.clear_and_free_semaphores(list(self.sems.allocated().values()))
    self.nc.all_engine_barrier()


tile.TileContext._drain_and_barrier = _split_drain_and_barrier

# Partition HWDGE DMA-completion lanes by issuing engine: SP (loads) cycles
# lanes 0-5, ACT (stores) alternates lanes 6-7. A DMA must wait for the
# previous DMA on its lane (sem-value determinism); with dedicated store
# lanes that predecessor is store(s-2), whose completion the evacuation
# "poke" already made ACT observe — so the wait elides and every store keeps
# a single sem wait (walrus limit).
import concourse.tile_sem_assignment as _tsa
from concourse import bass_isa as _bass_isa


def _assign_tick_lane_split(self, inst):
    engine = inst.engine
    eng_proc_idx = (
        _tsa.ENGINE_SEQUENCER_TO_IDX if inst.is_sequencer_only() else _tsa.ENGINE_TO_IDX
    )[engine]
    if isinstance(inst, _tsa.DMAInst) and not isinstance(
        inst, _bass_isa.UserSyncedRemoteDMADescs
    ):
        if engine == mybir.EngineType.Pool:
            inst_proc_idx = _tsa.PROC_NAME_TO_IDX[f"DMASW{self.next_sw_dma_idx}"]
            self.next_sw_dma_idx = (self.next_sw_dma_idx + 1) % self.swdge_sem_count
        elif engine == mybir.EngineType.Activation:
            n = getattr(self, "_act_dma_count", 0)
            inst_proc_idx = _tsa.PROC_NAME_TO_IDX[f"DMAHW{6 + (n % 2)}"]
            self._act_dma_count = n + 1
        else:
            inst_proc_idx = _tsa.PROC_NAME_TO_IDX[f"DMAHW{self.next_hw_dma_idx}"]
            self.next_hw_dma_idx = (self.next_hw_dma_idx + 1) % 6
    elif isinstance(inst, mybir.InstCollectiveCompute):
        inst_proc_idx = _tsa.PROC_NAME_TO_IDX["Collectives"]
    else:
        inst_proc_idx = eng_proc_idx

    if not inst.is_executable():
        if not isinstance(inst, _tsa.BassTileCriticalSection):
            return
    if isinstance(inst, _bass_isa.InstPseudoReloadLibraryIndex):
        return

    if inst.descendants or isinstance(inst, _tsa._DMA_OR_COLLECTIVE_TYPES):
        inst.bass_scheduled_tick = self.global_clock.advance(inst_proc_idx)
        inst.bass_scheduled_proc = inst_proc_idx
        inst.bass_scheduled_scope = self.scope_name
        self._proc_insts[self.root_scope_name][inst_proc_idx].append(inst)
        if getattr(inst, "gen_mode", 0) == 1 and inst_proc_idx != eng_proc_idx:
            eng_tick = self.global_clock.advance(eng_proc_idx)
            self.tc.prep_eng_ticks[inst.name] = (eng_proc_idx, eng_tick)
            self._prep_eng_names[self.root_scope_name].append(inst.name)


_tsa.TileClockTick._assign_tick = _assign_tick_lane_split

N_CORES = 8
H = W = 64
SLOT = 66                       # free-dim stride per image (64 data + 2 zero)
LEAD = 4                        # leading zero cols in a strip
S = 32                          # image pairs (slots) per strip
STRIP_W = LEAD + SLOT * S       # 2116 f32 per partition
N_STRIPS = 16                   # strips per core (16 * 64 = 1024 images)
# chunk = slot range processed by one PSUM bank (<=512 f32 out cols)
CHUNKS = [(0, 7), (7, 14), (14, 21), (21, 28), (28, 32)]

F32 = mybir.dt.float32
F32R = mybir.dt.float32r
STRIP_SELF_WAITS = False


def _chunk_geom(t0, t1):
    ns = t1 - t0
    n_cols = SLOT * (ns - 1) + 64          # contiguous out span incl. gaps
    o = LEAD + SLOT * t0                   # first data col of the chunk
    return ns, n_cols, o


def build_nc(n_strips: int, mm_dtype=F32R, relax: bool = True):
    """Build the Bass program for one core processing n_strips*64 images.

    Sync-topology note: walrus allows only ONE semaphore wait on most
    instruction structs (matmul/ldweights, DMA pseudo), so the program is
    shaped so every instruction has at most one cross-engine dependency:
      - each strip gets its own SBUF x tile -> loads have NO deps at all
        (pure prefetch, all queued on the SP HWDGE ring up front);
      - a tiny "absorber" matmul folds the load-DMA wait into PE program
        order; each chunk's first matmul carries its own single PSUM-WAR
        wait (previous occupant's ScalarE evacuation);
      - a 1-element ScalarE poke folds the out-buffer WAR (store of strip
        s-2) into ACT program order before the real evacuations, which also
        lets every store's lane-order wait elide.
    """
    from concourse.tile_rust import add_dep_helper as _adh
    from concourse.tile_scheduler import DMAInst

    def add_dep_helper(a, b, sync=False, reason=""):
        _adh(getattr(a, "ins", a), getattr(b, "ins", b), sync=sync, reason=reason)

    def relax_same_engine_deps(nc):
        """Demote same-engine compute->compute sync deps to order-only.

        Engines execute and complete their compute queues strictly in order,
        so a same-engine dependency never needs a semaphore — but Tile emits
        one anyway (self-waits), and walrus allows only a single sem wait on
        most instruction structs. DMA producers/consumers are excluded: a DMA
        instruction's completion is asynchronous to its issuing engine.
        """
        imap = nc.inst_map
        for inst in nc.all_instructions():
            if isinstance(inst, DMAInst) or not inst.is_executable():
                continue
            if inst.is_sequencer_only():
                continue
            sync_names = list(inst.sync_dependency_names())
            move = []
            for dn in sync_names:
                prod = imap.get(dn)
                if prod is None or isinstance(prod, DMAInst):
                    continue
                if not prod.is_executable() or prod.is_sequencer_only():
                    continue
                if prod.engine == inst.engine:
                    move.append(dn)
            if move:
                sync_set = inst.sync_dependency_set_copy()
                nosync_set = inst.nosync_dependency_set_copy()
                for dn in move:
                    sync_set.discard(dn)
                    nosync_set.add(dn)
                inst.set_sync_dependencies(sync_set)
                inst.set_nosync_dependencies(nosync_set)

    n_images = n_strips * 2 * S
    nc = bass.Bass(
        "TRN2", target_bir_lowering=False, detect_race_conditions=not relax
    )
    x_dram = nc.dram_tensor(
        "x", [n_strips, 128, STRIP_W], mm_dtype, kind="ExternalInput"
    )
    w_dram = nc.dram_tensor("w", [128, 512], mm_dtype, kind="ExternalInput")
    y_dram = nc.dram_tensor(
        "y", [n_strips, 128, 64 * S], F32, kind="ExternalOutput"
    )

    with tile.TileContext(nc) as tc:
        with (
            tc.tile_pool(name="pers", bufs=1) as pers,
            tc.tile_pool(name="psum", bufs=7, space="PSUM") as pp,
        ):
            wt = pers.tile([128, 512], mm_dtype, tag="wt")
            nc.sync.dma_start(wt[:], w_dram[:])

            x_tiles = [
                pers.tile([128, STRIP_W], mm_dtype, tag=f"xs{i}", name=f"xst{i}")
                for i in range(n_strips)
            ]
            y_bufs = [
                pers.tile([128, 64 * S], F32, tag=f"y{i}", name=f"ybuf{i}")
                for i in range(2)
            ]

            # prefetch every strip: no deps -> no waits, SP ring streams them
            for s in range(n_strips):
                nc.sync.dma_start(x_tiles[s][:], x_dram[s])

            # scratch PSUM tile for the absorber matmuls
            warm = pp.tile([128, 128], F32, name="warm", tag="warm", bufs=1)
            prev_mm = nc.tensor.matmul(
                warm[:], wt[:, 0:128], wt[:, 0:128], start=True, stop=True
            )

            for s in range(n_strips):
                xb = x_tiles[s]
                yb = y_bufs[s % 2]

                # absorber 1: load(s) completion -> PE program order
                d1 = nc.tensor.matmul(
                    warm[:, 0:4], wt[:, 0:128], xb[:, 0:4], start=True, stop=True
                )
                add_dep_helper(d1, prev_mm, sync=False, reason="strip order")
                gate = d1
                if s >= 1:
                    # absorber 2: strip s-1 PSUM evacuations (ACT) -> PE
                    # order. Reads the last column block chunk-2's copy
                    # wrote: with 7 PSUM banks and 5 chunks/strip, slot
                    # reuse reaches back at most to chunk-2 of the previous
                    # strip, so this covers the bank WARs while letting
                    # chunks 3-4's evacuations overlap the next strip's
                    # matmuls. Tile still emits exact per-chunk waits for
                    # anything this gate does not subsume.
                    pk = y_bufs[(s - 1) % 2][:, 64 * 21 - 4 : 64 * 21]
                    d2 = nc.tensor.matmul(
                        warm[0:4, 4:8], pk, pk, start=True, stop=True
                    )
                    add_dep_helper(d2, d1, sync=False, reason="absorber order")
                    gate = d2

                # ---- 4 banded matmuls per chunk, accumulated in PSUM ----
                psum_tiles = [
                    pp.tile([128, 512], F32, name=f"ps{s}_{ci}", tag="ps")
                    for ci in range(len(CHUNKS))
                ]
                first_mms = []
                for b in range(4):
                    lhsT = wt[:, 128 * b : 128 * (b + 1)]
                    d = 1 - b                      # horizontal tap shift
                    for ci, (t0, t1) in enumerate(CHUNKS):
                        ns, n_cols, o = _chunk_geom(t0, t1)
                        rhs = xb[:, o + d : o + d + n_cols]
                        mm = nc.tensor.matmul(
                            psum_tiles[ci][:, 0:n_cols],
                            lhsT,
                            rhs,
                            start=(b == 0),
                            stop=(b == 3),
                        )
                        if b == 0:
                            add_dep_helper(mm, gate, sync=False, reason="gate")
                        prev_mm = mm

                # absorber 3: store(s-2) completion -> ACT program order.
                # Pokes one element of yb (chunk-0's copy rewrites it next).
                d3 = nc.scalar.copy(yb[0:1, 0:1], wt[0:1, 0:1].bitcast(F32))

                # ---- evacuate PSUM -> packed out tile (VectorE) ----
                copies = []
                for ci, (t0, t1) in enumerate(CHUNKS):
                    ns, n_cols, o = _chunk_geom(t0, t1)
                    src_c = psum_tiles[ci][:, 0 : SLOT * ns].rearrange(
                        "p (t u) -> p t u", u=SLOT
                    )[:, :, 0:64]
                    dst_c = yb[:, 64 * t0 : 64 * t1].rearrange(
                        "p (t w) -> p t w", w=64
                    )
                    cp = nc.scalar.copy(dst_c, src_c)
                    add_dep_helper(cp, d3, sync=False, reason="poke order")
                    copies.append(cp)

                # ---- store: dense permuted dump (host inverse-permutes) ----
                nc.scalar.dma_start(y_dram[s], yb[:])

            if relax:
                relax_same_engine_deps(nc)

    if relax and STRIP_SELF_WAITS:
        _strip_self_satisfied_waits(nc)

    return nc


def _strip_self_satisfied_waits(nc):
    """Post-scheduling: drop sem waits already guaranteed by the issuing
    engine's own instruction stream (e.g. PE waiting on the PE semaphore for
    a PSUM-slot WAW against its own earlier matmuls — the pool allocator
    emits these during scheduling, after the dep-relaxation pass ran).

    Safe because an engine's compute instructions complete in stream order,
    and only increments issued synchronously by THIS engine's earlier
    non-DMA instructions are counted (DMA completions are asynchronous and
    excluded). Walrus allows one sem wait per instruction, so these
    redundant self-waits are the difference between compiling and not.
    """
    from concourse.tile_scheduler import DMAInst

    cum: dict = {}
    for inst in nc.all_instructions():
        si = inst.sync_info
        if si is None:
            continue
        c = cum.setdefault(str(inst.engine), {})
        waits = list(si.on_wait)
        keep = [
            w
            for w in waits
            if not (
                w.sync_type == "semaphore"
                and w.wait_mode == "sem-ge-imm"
                and w.wait_reg is None
                and c.get(w.ant_name, 0) >= w.wait_value
            )
        ]
        if len(keep) != len(waits):
            si.on_wait = keep
        if not isinstance(inst, DMAInst):
            for u in si.on_update:
                if u.sync_type == "semaphore" and u.update_mode == "sem-inc":
                    c[u.ant_name] = c.get(u.ant_name, 0) + (u.update_value or 1)


def build_weights(kern: np.ndarray) -> np.ndarray:
    """4 banded lhsT matrices [K=128(in row), M=128(out row)], one per
    horizontal tap b: lhsT_b[k, m] = kern[m+1-k, b]; block-diag per image."""
    kern = np.asarray(kern, np.float32)
    w = np.zeros((128, 4 * 128), np.float32)
    for b in range(4):
        for blk in (0, 64):
            for m in range(64):
                for a in range(4):
                    k = m + 1 - a
                    if 0 <= k < 64:
                        w[blk + k, 128 * b + blk + m] = kern[a, b]
    return w


def marshal(x: np.ndarray, n_cores: int = N_CORES) -> np.ndarray:
    """Full (G, 64, 64) f32 -> prepadded per-core strips
    [n_cores, N_STRIPS, 128, STRIP_W]."""
    G = x.shape[0]
    n_strips = G // (n_cores * 2 * S)
    xr = x.reshape(n_cores, n_strips, S, 2, H, W)          # [c, s, t, j, r, w]
    out = np.zeros((n_cores, n_strips, 128, STRIP_W), np.float32)
    view = out[:, :, :, LEAD : LEAD + SLOT * S].reshape(
        n_cores, n_strips, 2, H, S, SLOT
    )                                                       # [c, s, j, r, t, u]
    view[..., 0:64] = xr.transpose(0, 1, 3, 4, 2, 5)
    return out


def unmarshal_y(yp: np.ndarray) -> np.ndarray:
    """Per-core permuted output [n_cores, N_STRIPS, 128, 64*S] -> (G, 64, 64)."""
    n_cores, n_strips = yp.shape[0], yp.shape[1]
    v = yp.reshape(n_cores, n_strips, 2, H, S, 64)         # [c, s, j, r, t, w]
    return np.ascontiguousarray(
        v.transpose(0, 1, 4, 2, 3, 5)                      # [c, s, t, j, r, w]
    ).reshape(n_cores * n_strips * 2 * S, H, W)


def make_in_maps(x: np.ndarray, kern: np.ndarray):
    """x: (B, C, 64, 64) f32 -> per-core input maps."""
    G = x.shape[0] * x.shape[1]
    xp = marshal(x.reshape(G, H, W))
    w_all = build_weights(kern)
    return [{"x": xp[c], "w": w_all} for c in range(N_CORES)]


_CACHE: dict = {}


def _get_nc():
    if "nc" not in _CACHE:
        _CACHE["nc"] = build_nc(n_strips=N_STRIPS)
    return _CACHE["nc"]


def kernel(x, kernel):
    x = np.ascontiguousarray(np.asarray(x, dtype=np.float32))
    kern = np.asarray(kernel, dtype=np.float32)
    B, C, HH, WW = x.shape

    nc = _get_nc()
    in_maps = make_in_maps(x, kern)
    res = run_bass_kernel_spmd(nc, in_maps, list(range(N_CORES)))
    yp = np.stack([res.results[c]["y"] for c in range(N_CORES)], axis=0)
    return unmarshal_y(yp).reshape(B, C, HH, WW).astype(np.float32)


if __name__ == "__main__":
    # quick self-check against numpy on random data (runs on hardware)
    rng = np.random.default_rng(0)
    x = rng.standard_normal((16, 512, 64, 64), dtype=np.float32)
    k1 = np.array([1.0, 3.0, 3.0, 1.0], np.float32)
    kern = np.outer(k1, k1)
    kern /= kern.sum()
    y = kernel(x, kern)
    print("out shape", y.shape, "dtype", y.dtype)



# revision 18
# speedup vs baseline: 1.7423x; 1.7423x over previous
"""Depthwise 4x4 FIR blur (upfirdn2d-style) on 8 Trainium2 NeuronCores.

Input  x: (16, 512, 64, 64) f32, kernel: (4, 4) f32 (normalized binomial).
Output y: same shape as x, y[g] = conv2d(zero-pad(x[g], (2,1)x(2,1)), flip(kernel)).

Equivalent per-image formula (derived from the reference):
    y[i, j] = sum_{a,b in [0,4)} kernel[a, b] * x[i+1-a, j+1-b]   (zero outside)

v2 strategy (fp16 I/O + separable factorization, ~2x over the all-matmul v1):
  - Device I/O is fp16 (tolerance is 2e-2; measured chain error ~8e-4), which
    halves HBM traffic to ~17 MB/core: 16 strips of [128, 2116] in, 16 of
    [128, 2048] out. Host pre-pads strips (2 zero cols between images, 4 lead
    zeros) so horizontal taps are free-dim shifts that read zeros across
    image boundaries; partition k<64 = even image rows, k>=64 = odd.
  - The kernel is separable and binomial: K = outer(Vw, [1,3,3,1]) with
    Vw = K[:,0], and [1,3,3,1] = [1,1] (*) [1,1] (*) [1,1]. Work splits as:
      PE:  w = (vertical-band V . x) (*)_h [1,1]  -- 2 matmuls per PSUM chunk
           (identical lhsT), 10 matmuls/strip instead of v1's 20+absorbers.
      ACT: v0 = fp16(w)  PSUM -> SBUF dense copy (3 copies/strip).
      DVE: v1 = v0 + shift1(v0); y = v1 + shift1(v1)  -- fp16 adds at 2x
           throughput, writing the packed [128, 2048] out tile; DVE also
           issues the store.
  - PSUM: chunks of 32 slots split (7,7,7,7,4); pair-tiles p01/p4 double-
    buffered, p23 single-buffered = exactly 8 banks. PE emits chunk 4 first
    and ACT copies in order (4, 01, 23) so every PSUM-WAR wait is subsumed
    by an earlier, larger-valued wait on the same semaphore (walrus allows
    only one sem wait per matmul) -- no absorber matmuls needed.
  - Load DMAs (SP) cycle HWDGE lanes 0-3, store DMAs (DVE) lanes 4-7;
    1-element pokes fold buffer-WAR waits into engine program order so
    every store's lane-order wait elides (same pattern as v1).
"""

import numpy as np

import concourse.bass as bass
import concourse.tile as tile
from concourse import mybir
from concourse.bass_utils import run_bass_kernel_spmd

# The kernel-tail drain waits on every semaphore family the kernel touched
# (PE + ACT + up to 8 DMA lanes); walrus rejects instructions with that many
# sync waits. Split the drain into several drain instructions, each carrying
# at most 3 waits — semantically identical (SP executes them in sequence).
import bass_rust as _bass_rust
from concourse.tile_scheduler import N_PROCS as _N_PROCS

def _split_drain_and_barrier(self, tick_clock, wait_clock):
    ScopedClock = _bass_rust.ScopedClock
    VectorClock = _bass_rust.VectorClock
    gc = tick_clock.global_clock
    vals = [gc[p] for p in range(_N_PROCS)]
    nonzero = [p for p in range(_N_PROCS) if vals[p] > 0]
    for p in nonzero:
        pv = [vals[q] if q == p else 0 for q in range(_N_PROCS)]
        d = self.nc.sync.drain()
        wait_clock.add_sem_waits(d.ins, ScopedClock({None: VectorClock(pv)}))
    self.nc.sync.drain()

    self.nc.all_engine_barrier()
    assert self.sems is not None
    popped = self.nc._tile_sem_poison_stack.pop()
    assert popped is self._sem_poison
    self.nc.clear_and_free_semaphores(list(self.sems.allocated().values()))
    self.nc.all_engine_barrier()


tile.TileContext._drain_and_barrier = _split_drain_and_barrier

# Partition HWDGE DMA-completion lanes by issuing engine: SP (loads) cycles
# lanes 0-3, ACT (stores) cycles lanes 4-7. A DMA must wait for the previous
# DMA on its lane (sem-value determinism); with dedicated store lanes that
# predecessor is store(s-4), whose completion the ACT store-poke of strip
# s-2 already made ACT observe — so the wait elides and every store keeps a
# single sem wait (walrus limit).
import concourse.tile_sem_assignment as _tsa
from concourse import bass_isa as _bass_isa


def _assign_tick_lane_split(self, inst):
    engine = inst.engine
    eng_proc_idx = (
        _tsa.ENGINE_SEQUENCER_TO_IDX if inst.is_sequencer_only() else _tsa.ENGINE_TO_IDX
    )[engine]
    if isinstance(inst, _tsa.DMAInst) and not isinstance(
        inst, _bass_isa.UserSyncedRemoteDMADescs
    ):
        if engine == mybir.EngineType.Pool:
            inst_proc_idx = _tsa.PROC_NAME_TO_IDX[f"DMASW{self.next_sw_dma_idx}"]
            self.next_sw_dma_idx = (self.next_sw_dma_idx + 1) % self.swdge_sem_count
        elif engine == mybir.EngineType.Activation:
            n = getattr(self, "_act_dma_count", 0)
            inst_proc_idx = _tsa.PROC_NAME_TO_IDX[f"DMAHW{4 + (n % 4)}"]
            self._act_dma_count = n + 1
        else:
            inst_proc_idx = _tsa.PROC_NAME_TO_IDX[f"DMAHW{self.next_hw_dma_idx}"]
            self.next_hw_dma_idx = (self.next_hw_dma_idx + 1) % 4
    elif isinstance(inst, mybir.InstCollectiveCompute):
        inst_proc_idx = _tsa.PROC_NAME_TO_IDX["Collectives"]
    else:
        inst_proc_idx = eng_proc_idx

    if not inst.is_executable():
        if not isinstance(inst, _tsa.BassTileCriticalSection):
            return
    if isinstance(inst, _bass_isa.InstPseudoReloadLibraryIndex):
        return

    if inst.descendants or isinstance(inst, _tsa._DMA_OR_COLLECTIVE_TYPES):
        inst.bass_scheduled_tick = self.global_clock.advance(inst_proc_idx)
        inst.bass_scheduled_proc = inst_proc_idx
        inst.bass_scheduled_scope = self.scope_name
        self._proc_insts[self.root_scope_name][inst_proc_idx].append(inst)
        if getattr(inst, "gen_mode", 0) == 1 and inst_proc_idx != eng_proc_idx:
            eng_tick = self.global_clock.advance(eng_proc_idx)
            self.tc.prep_eng_ticks[inst.name] = (eng_proc_idx, eng_tick)
            self._prep_eng_names[self.root_scope_name].append(inst.name)


_tsa.TileClockTick._assign_tick = _assign_tick_lane_split

N_CORES = 8
H = W = 64
SLOT = 66                       # free-dim stride per image (64 data + 2 zero)
LEAD = 4                        # leading zero cols in a strip
S = 32                          # image pairs (slots) per strip
STRIP_W = LEAD + SLOT * S       # 2116 elements per partition
N_STRIPS = 16                   # strips per core (16 * 64 = 1024 images)
# chunks of slots per PSUM bank; mm width 66*ns <= 512 f32
CHUNK_NS = [7, 7, 7, 7, 4]
CHUNK_T0 = [0, 7, 14, 21, 28]
V0_W = SLOT * S                 # 2112: dense w-range [2, 2114)

F16 = mybir.dt.float16
F32 = mybir.dt.float32


def build_nc(n_strips: int = N_STRIPS, relax: bool = True):
    """Build the Bass program for one core processing n_strips*64 images.

    Sync-topology: every instruction carries at most one semaphore wait.
      - per-strip SBUF x tiles -> loads are pure prefetch with no waits;
      - a single ldweights absorber folds the wt-load wait into PE order;
      - PE chunk order (4,0,1,2,3) + ACT copy order (4,01,23) make each
        PSUM-WAR wait either the single wait on the chunk's first matmul or
        already subsumed by a previous larger wait on the ACT semaphore;
      - 1-element pokes pre-observe cross-engine buffer WARs (ACT: v0 vs
        DVE v1-add of strip s-2; DVE: yb vs store of strip s-2).
    """
    from concourse.tile_rust import add_dep_helper as _adh
    from concourse.tile_scheduler import DMAInst

    def add_dep_helper(a, b, sync=False, reason=""):
        _adh(getattr(a, "ins", a), getattr(b, "ins", b), sync=sync, reason=reason)

    def relax_same_engine_deps(nc):
        """Demote same-engine compute->compute sync deps to order-only.

        Engines execute and complete their compute queues strictly in order,
        so a same-engine dependency never needs a semaphore — but Tile emits
        one anyway (self-waits), and walrus allows only a single sem wait on
        most instruction structs. DMA producers/consumers are excluded: a DMA
        instruction's completion is asynchronous to its issuing engine.
        """
        imap = nc.inst_map
        for inst in nc.all_instructions():
            if isinstance(inst, DMAInst) or not inst.is_executable():
                continue
            if inst.is_sequencer_only():
                continue
            sync_names = list(inst.sync_dependency_names())
            move = []
            for dn in sync_names:
                prod = imap.get(dn)
                if prod is None or isinstance(prod, DMAInst):
                    continue
                if not prod.is_executable() or prod.is_sequencer_only():
                    continue
                if prod.engine == inst.engine:
                    move.append(dn)
            if move:
                sync_set = inst.sync_dependency_set_copy()
                nosync_set = inst.nosync_dependency_set_copy()
                for dn in move:
                    sync_set.discard(dn)
                    nosync_set.add(dn)
                inst.set_sync_dependencies(sync_set)
                inst.set_nosync_dependencies(nosync_set)

    def tensor_tensor(eng, out, in0, in1):
        """Plain 2-tensor elementwise add on DVE/Pool (InstTensorTensor gets
        the 2x 16-bit DVE mode; scalar_tensor_tensor does not)."""
        return eng.add_instruction(
            mybir.InstTensorTensor(
                name=nc.get_next_instruction_name(),
                op=mybir.AluOpType.add,
                ins=[eng.lower_ap(in0), eng.lower_ap(in1)],
                outs=[eng.lower_ap(out)],
            )
        )

    nc = bass.Bass(
        "TRN2", target_bir_lowering=False, detect_race_conditions=not relax
    )
    x_dram = nc.dram_tensor(
        "x", [n_strips, 128, STRIP_W], F16, kind="ExternalInput"
    )
    w_dram = nc.dram_tensor("w", [128, 128], F16, kind="ExternalInput")
    y_dram = nc.dram_tensor(
        "y", [n_strips, 128, 64 * S], F16, kind="ExternalOutput"
    )

    with tile.TileContext(nc) as tc:
        with (
            tc.tile_pool(name="pers", bufs=1) as pers,
            tc.tile_pool(name="psum", bufs=2, space="PSUM") as pp,
        ):
            wt = pers.tile([128, 128], F16, tag="wt")
            nc.sync.dma_start(wt[:], w_dram[:])

            x_tiles = [
                pers.tile([128, STRIP_W], F16, tag=f"xs{i}", name=f"xst{i}")
                for i in range(n_strips)
            ]
            v0_bufs = [
                pers.tile([128, V0_W], F16, tag=f"v0_{i}", name=f"v0b{i}")
                for i in range(2)
            ]
            v1_bufs = [
                pers.tile([128, V0_W], F16, tag=f"v1_{i}", name=f"v1b{i}")
                for i in range(2)
            ]
            y_bufs = [
                pers.tile([128, 64 * S], F16, tag=f"y{i}", name=f"ybuf{i}")
                for i in range(2)
            ]
            # dedicated poke scratch: pokes only need to make their engine
            # OBSERVE a store-completion semaphore, not touch real buffers
            pk_d = pers.tile([128, 2], F16, tag="pk_d")
            pk_a = pers.tile([128, 2], F16, tag="pk_a")

            # prefetch every strip: no deps -> no waits, SP ring streams them
            load_insts = []
            for s in range(n_strips):
                load_insts.append(nc.sync.dma_start(x_tiles[s][:], x_dram[s]))

            # absorber: folds the wt-load wait into PE program order so no
            # matmul carries it (they each have their own single WAR wait)
            nc.tensor.ldweights(wt[:])

            store_insts = []
            for s in range(n_strips):
                xb = x_tiles[s]
                v0 = v0_bufs[s % 2]
                v1 = v1_bufs[s % 2]
                yb = y_bufs[s % 2]

                p01 = pp.tile([128, 1024], F32, tag="p01", bufs=2, name=f"p01_{s}")
                p23 = pp.tile([128, 1024], F32, tag="p23", bufs=1, name=f"p23_{s}")
                p4 = pp.tile([128, 512], F32, tag="p4", bufs=2, name=f"p4_{s}")

                def psum_slice(k):
                    w = SLOT * CHUNK_NS[k]
                    if k < 4:
                        t = p01 if k < 2 else p23
                        off = 512 * (k % 2)
                        return t[:, off : off + w]
                    return p4[:, 0:w]

                # per-strip absorber: folds the xb-load wait into PE program
                # order so each chunk's first matmul carries only its single
                # PSUM-WAR wait (walrus allows one sem wait per matmul)
                ldw = nc.tensor.ldweights(wt[:])
                add_dep_helper(ldw, load_insts[s], sync=True, reason="x load")

                # ---- PE: w = (V.x) (*)_h [1,1], chunk 4 first ----
                # chunk k covers w-positions [2+66*t0, +66*ns); tap e reads
                # xb cols shifted by e.
                for k in (4, 0, 1, 2, 3):
                    t0, ns = CHUNK_T0[k], CHUNK_NS[k]
                    base = 2 + SLOT * t0
                    wk = SLOT * ns
                    dst = psum_slice(k)
                    for e in (0, 1):
                        nc.tensor.matmul(
                            dst,
                            wt[:],
                            xb[:, base + e : base + e + wk],
                            start=(e == 0),
                            stop=(e == 1),
                        )

                # ---- ACT: v0 = fp16(w), order (4, 01, 23) ----
                # poke folds the v0-buffer WAR (DVE v1-add of strip s-2)
                # into ACT program order.
                nc.scalar.copy(v0[0:1, 0:1], wt[0:1, 0:1])
                nc.scalar.copy(
                    v0[:, SLOT * 28 : V0_W], p4[:, 0 : SLOT * 4]
                )
                nc.scalar.copy(
                    v0[:, 0 : 2 * 462].rearrange("p (a b) -> p a b", b=462),
                    p01[:].rearrange("p (a b) -> p a b", b=512)[:, :, 0:462],
                )
                nc.scalar.copy(
                    v0[:, 2 * 462 : 4 * 462].rearrange("p (a b) -> p a b", b=462),
                    p23[:].rearrange("p (a b) -> p a b", b=512)[:, :, 0:462],
                )

                # ---- DVE: two fp16 [1,1] passes ----
                # poke makes DVE observe store(s-2) completion so the y-add's
                # yb WAR wait elides.
                dpk = nc.vector.memset(pk_d[0:1, 0:1], 0.0)
                if s >= 2:
                    add_dep_helper(
                        dpk, store_insts[s - 2], sync=True, reason="yb war"
                    )
                tensor_tensor(
                    nc.vector,
                    v1[:, 0 : V0_W - 1],
                    v0[:, 0 : V0_W - 1],
                    v0[:, 1:V0_W],
                )
                v1s = v1[:].rearrange("p (t u) -> p t u", u=SLOT)
                dst = yb[:].rearrange("p (t w) -> p t w", w=64)
                tensor_tensor(nc.vector, dst, v1s[:, :, 0:64], v1s[:, :, 1:65])

                # ---- store: dense permuted dump (host inverse-permutes) ----
                # ACT poke observes store(s-2) so store(s)'s lane-order wait
                # (on store(s-4), seen by the s-2 poke) elides.
                apk = nc.scalar.copy(pk_a[0:1, 0:1], wt[0:1, 0:1])
                if s >= 2:
                    add_dep_helper(
                        apk, store_insts[s - 2], sync=True, reason="lane order"
                    )
                st = nc.scalar.dma_start(y_dram[s], yb[:])
                store_insts.append(st)

            if relax:
                relax_same_engine_deps(nc)

    if relax:
        _strip_self_satisfied_waits(nc)

    return nc


def _strip_self_satisfied_waits(nc):
    """Post-scheduling: drop sem waits already guaranteed by the issuing
    engine's own instruction stream (e.g. PE waiting on the PE semaphore for
    a PSUM-slot WAW against its own earlier matmuls — the pool allocator
    emits these during scheduling, after the dep-relaxation pass ran).

    Safe because an engine's compute instructions complete in stream order,
    and only increments issued synchronously by THIS engine's earlier
    non-DMA instructions are counted (DMA completions are asynchronous and
    excluded). Walrus allows one sem wait per instruction, so these
    redundant self-waits are the difference between compiling and not.
    """
    from concourse.tile_scheduler import DMAInst

    cum: dict = {}
    for inst in nc.all_instructions():
        si = inst.sync_info
        if si is None:
            continue
        c = cum.setdefault(str(inst.engine), {})
        waits = list(si.on_wait)
        keep = [
            w
            for w in waits
            if not (
                w.sync_type == "semaphore"
                and w.wait_mode == "sem-ge-imm"
                and w.wait_reg is None
                and c.get(w.ant_name, 0) >= w.wait_value
            )
        ]
        if len(keep) != len(waits):
            si.on_wait = keep
        if not isinstance(inst, DMAInst):
            for u in si.on_update:
                if u.sync_type == "semaphore" and u.update_mode == "sem-inc":
                    c[u.ant_name] = c.get(u.ant_name, 0) + (u.update_value or 1)


def build_weights(kern: np.ndarray) -> np.ndarray:
    """Vertical banded lhsT [K=128(in row), M=128(out row)], block-diag per
    image: V[64j + r', 64j + r] = Vw[r+1-r'] with Vw = kern[:, 0]; the
    horizontal [1,3,3,1] factor is applied by the [1,1] tap pair + two DVE
    add passes."""
    kern = np.asarray(kern, np.float32)
    Vw = kern[:, 0]
    h = kern[0, :] / kern[0, 0]
    assert np.allclose(h, [1.0, 3.0, 3.0, 1.0], atol=1e-5), h
    assert np.allclose(kern, np.outer(Vw, h), atol=1e-7)
    v = np.zeros((128, 128), np.float32)
    for blk in (0, 64):
        for r in range(64):
            for a in range(4):
                rp = r + 1 - a
                if 0 <= rp < 64:
                    v[blk + rp, blk + r] = Vw[a]
    return v.astype(np.float16)


def marshal(x: np.ndarray, n_cores: int = N_CORES) -> np.ndarray:
    """Full (G, 64, 64) f32 -> prepadded per-core fp16 strips
    [n_cores, N_STRIPS, 128, STRIP_W]."""
    G = x.shape[0]
    n_strips = G // (n_cores * 2 * S)
    xr = x.reshape(n_cores, n_strips, S, 2, H, W)          # [c, s, t, j, r, w]
    out = np.zeros((n_cores, n_strips, 128, STRIP_W), np.float16)
    view = out[:, :, :, LEAD : LEAD + SLOT * S].reshape(
        n_cores, n_strips, 2, H, S, SLOT
    )                                                       # [c, s, j, r, t, u]
    view[..., 0:64] = xr.transpose(0, 1, 3, 4, 2, 5)
    return out


def unmarshal_y(yp: np.ndarray) -> np.ndarray:
    """Per-core permuted output [n_cores, N_STRIPS, 128, 64*S] fp16 ->
    (G, 64, 64) f32."""
    n_cores, n_strips = yp.shape[0], yp.shape[1]
    v = yp.reshape(n_cores, n_strips, 2, H, S, 64)         # [c, s, j, r, t, w]
    return np.ascontiguousarray(
        v.transpose(0, 1, 4, 2, 3, 5)                      # [c, s, t, j, r, w]
    ).astype(np.float32).reshape(n_cores * n_strips * 2 * S, H, W)


def make_in_maps(x: np.ndarray, kern: np.ndarray):
    """x: (B, C, 64, 64) f32 -> per-core input maps."""
    G = x.shape[0] * x.shape[1]
    xp = marshal(np.asarray(x, np.float32).reshape(G, H, W))
    w_all = build_weights(kern)
    return [{"x": xp[c], "w": w_all} for c in range(N_CORES)]


_CACHE: dict = {}


def _get_nc():
    if "nc" not in _CACHE:
        _CACHE["nc"] = build_nc(n_strips=N_STRIPS)
    return _CACHE["nc"]


def kernel(x, kernel):
    x = np.ascontiguousarray(np.asarray(x, dtype=np.float32))
    kern = np.asarray(kernel, dtype=np.float32)
    B, C, HH, WW = x.shape

    nc = _get_nc()
    in_maps = make_in_maps(x, kern)
    res = run_bass_kernel_spmd(nc, in_maps, list(range(N_CORES)))
    yp = np.stack([res.results[c]["y"] for c in range(N_CORES)], axis=0)
    return unmarshal_y(yp).reshape(B, C, HH, WW).astype(np.float32)


if __name__ == "__main__":
    # quick self-check against numpy on random data (runs on hardware)
    rng = np.random.default_rng(0)
    x = rng.standard_normal((16, 512, 64, 64), dtype=np.float32)
    k1 = np.array([1.0, 3.0, 3.0, 1.0], np.float32)
    kern = np.outer(k1, k1)
    kern /= kern.sum()
    y = kernel(x, kern)
    print("out shape", y.shape, "dtype", y.dtype)


# revision 34
# speedup vs baseline: 1.9842x; 1.1388x over previous
"""Depthwise 4x4 FIR blur (upfirdn2d-style) on 8 Trainium2 NeuronCores.

Input  x: (16, 512, 64, 64) f32, kernel: (4, 4) f32 (normalized binomial).
Output y: same shape as x, y[g] = conv2d(zero-pad(x[g], (2,1)x(2,1)), flip(kernel)).

Equivalent per-image formula (derived from the reference):
    y[i, j] = sum_{a,b in [0,4)} kernel[a, b] * x[i+1-a, j+1-b]   (zero outside)

v2 strategy (fp16 I/O + separable factorization, ~2x over the all-matmul v1):
  - Device I/O is fp16 (tolerance is 2e-2; measured chain error ~8e-4), which
    halves HBM traffic to ~17 MB/core: 16 strips of [128, 2116] in, 16 of
    [128, 2048] out. Host pre-pads strips (2 zero cols between images, 4 lead
    zeros) so horizontal taps are free-dim shifts that read zeros across
    image boundaries; partition k<64 = even image rows, k>=64 = odd.
  - The kernel is separable and binomial: K = outer(Vw, [1,3,3,1]) with
    Vw = K[:,0], and [1,3,3,1] = [1,1] (*) [1,1] (*) [1,1]. Work splits as:
      PE:  w = (vertical-band V . x) (*)_h [1,1]  -- 2 matmuls per PSUM chunk
           (identical lhsT), 10 matmuls/strip instead of v1's 20+absorbers.
      ACT: v0 = fp16(w)  PSUM -> SBUF dense copy (3 copies/strip).
      DVE: v1 = v0 + shift1(v0); y = v1 + shift1(v1)  -- fp16 adds at 2x
           throughput, writing the packed [128, 2048] out tile; DVE also
           issues the store.
  - PSUM: chunks of 32 slots split (7,7,7,7,4); pair-tiles p01/p4 double-
    buffered, p23 single-buffered = exactly 8 banks. PE emits chunk 4 first
    and ACT copies in order (4, 01, 23) so every PSUM-WAR wait is subsumed
    by an earlier, larger-valued wait on the same semaphore (walrus allows
    only one sem wait per matmul) -- no absorber matmuls needed.
  - Load DMAs (SP) cycle HWDGE lanes 0-3, store DMAs (DVE) lanes 4-7;
    1-element pokes fold buffer-WAR waits into engine program order so
    every store's lane-order wait elides (same pattern as v1).
"""

import numpy as np

import concourse.bass as bass
import concourse.tile as tile
from concourse import mybir
from concourse.bass_utils import run_bass_kernel_spmd

# The kernel-tail drain waits on every semaphore family the kernel touched
# (PE + ACT + up to 8 DMA lanes); walrus rejects instructions with that many
# sync waits. Split the drain into several drain instructions, each carrying
# at most 3 waits — semantically identical (SP executes them in sequence).
import bass_rust as _bass_rust
from concourse.tile_scheduler import N_PROCS as _N_PROCS

def _split_drain_and_barrier(self, tick_clock, wait_clock):
    ScopedClock = _bass_rust.ScopedClock
    VectorClock = _bass_rust.VectorClock
    gc = tick_clock.global_clock
    vals = [gc[p] for p in range(_N_PROCS)]
    nonzero = [p for p in range(_N_PROCS) if vals[p] > 0]
    for p in nonzero:
        pv = [vals[q] if q == p else 0 for q in range(_N_PROCS)]
        d = self.nc.sync.drain()
        wait_clock.add_sem_waits(d.ins, ScopedClock({None: VectorClock(pv)}))
    self.nc.sync.drain()

    self.nc.all_engine_barrier()
    assert self.sems is not None
    popped = self.nc._tile_sem_poison_stack.pop()
    assert popped is self._sem_poison
    self.nc.clear_and_free_semaphores(list(self.sems.allocated().values()))
    self.nc.all_engine_barrier()


tile.TileContext._drain_and_barrier = _split_drain_and_barrier

# Partition HWDGE DMA-completion lanes by issuing engine: SP (loads) cycles
# lanes 0-3, ACT (stores) cycles lanes 4-7. A DMA must wait for the previous
# DMA on its lane (sem-value determinism); with dedicated store lanes that
# predecessor is store(s-4), whose completion the ACT store-poke of strip
# s-2 already made ACT observe — so the wait elides and every store keeps a
# single sem wait (walrus limit).
import concourse.tile_sem_assignment as _tsa
from concourse import bass_isa as _bass_isa


def _assign_tick_lane_split(self, inst):
    engine = inst.engine
    eng_proc_idx = (
        _tsa.ENGINE_SEQUENCER_TO_IDX if inst.is_sequencer_only() else _tsa.ENGINE_TO_IDX
    )[engine]
    if isinstance(inst, _tsa.DMAInst) and not isinstance(
        inst, _bass_isa.UserSyncedRemoteDMADescs
    ):
        if engine == mybir.EngineType.Pool:
            inst_proc_idx = _tsa.PROC_NAME_TO_IDX[f"DMASW{self.next_sw_dma_idx}"]
            self.next_sw_dma_idx = (self.next_sw_dma_idx + 1) % self.swdge_sem_count
        elif engine == mybir.EngineType.Activation:
            n = getattr(self, "_act_dma_count", 0)
            inst_proc_idx = _tsa.PROC_NAME_TO_IDX[f"DMAHW{4 + (n % 4)}"]
            self._act_dma_count = n + 1
        else:
            inst_proc_idx = _tsa.PROC_NAME_TO_IDX[f"DMAHW{self.next_hw_dma_idx}"]
            self.next_hw_dma_idx = (self.next_hw_dma_idx + 1) % 4
    elif isinstance(inst, mybir.InstCollectiveCompute):
        inst_proc_idx = _tsa.PROC_NAME_TO_IDX["Collectives"]
    else:
        inst_proc_idx = eng_proc_idx

    if not inst.is_executable():
        if not isinstance(inst, _tsa.BassTileCriticalSection):
            return
    if isinstance(inst, _bass_isa.InstPseudoReloadLibraryIndex):
        return

    if inst.descendants or isinstance(inst, _tsa._DMA_OR_COLLECTIVE_TYPES):
        inst.bass_scheduled_tick = self.global_clock.advance(inst_proc_idx)
        inst.bass_scheduled_proc = inst_proc_idx
        inst.bass_scheduled_scope = self.scope_name
        self._proc_insts[self.root_scope_name][inst_proc_idx].append(inst)
        if getattr(inst, "gen_mode", 0) == 1 and inst_proc_idx != eng_proc_idx:
            eng_tick = self.global_clock.advance(eng_proc_idx)
            self.tc.prep_eng_ticks[inst.name] = (eng_proc_idx, eng_tick)
            self._prep_eng_names[self.root_scope_name].append(inst.name)


_tsa.TileClockTick._assign_tick = _assign_tick_lane_split

N_CORES = 8
H = W = 64
SLOT = 66                       # free-dim stride per image (64 data + 2 zero)
LEAD = 4                        # leading zero cols in a strip
S = 32                          # image pairs (slots) per strip
STRIP_W = LEAD + SLOT * S       # 2116 elements per partition
N_STRIPS = 16                   # strips per core (16 * 64 = 1024 images)
# chunks of slots per PSUM bank; mm width 66*ns <= 512 f32
CHUNK_NS = [7, 7, 7, 7, 4]
CHUNK_T0 = [0, 7, 14, 21, 28]
V0_W = SLOT * S                 # 2112: dense w-range [2, 2114)

F16 = mybir.dt.float16
F32 = mybir.dt.float32


def build_nc(n_strips: int = N_STRIPS, relax: bool = True):
    """Build the Bass program for one core processing n_strips*64 images.

    Sync-topology: every instruction carries at most one semaphore wait.
      - per-strip SBUF x tiles -> loads are pure prefetch with no waits;
      - a single ldweights absorber folds the wt-load wait into PE order;
      - PE chunk order (4,0,1,2,3) + ACT copy order (4,01,23) make each
        PSUM-WAR wait either the single wait on the chunk's first matmul or
        already subsumed by a previous larger wait on the ACT semaphore;
      - 1-element pokes pre-observe cross-engine buffer WARs (ACT: v0 vs
        DVE v1-add of strip s-2; DVE: yb vs store of strip s-2).
    """
    from concourse.tile_rust import add_dep_helper as _adh
    from concourse.tile_scheduler import DMAInst

    def add_dep_helper(a, b, sync=False, reason=""):
        _adh(getattr(a, "ins", a), getattr(b, "ins", b), sync=sync, reason=reason)

    def relax_same_engine_deps(nc):
        """Demote same-engine compute->compute sync deps to order-only.

        Engines execute and complete their compute queues strictly in order,
        so a same-engine dependency never needs a semaphore — but Tile emits
        one anyway (self-waits), and walrus allows only a single sem wait on
        most instruction structs. DMA producers/consumers are excluded: a DMA
        instruction's completion is asynchronous to its issuing engine.
        """
        imap = nc.inst_map
        for inst in nc.all_instructions():
            if isinstance(inst, DMAInst) or not inst.is_executable():
                continue
            if inst.is_sequencer_only():
                continue
            sync_names = list(inst.sync_dependency_names())
            move = []
            for dn in sync_names:
                prod = imap.get(dn)
                if prod is None or isinstance(prod, DMAInst):
                    continue
                if not prod.is_executable() or prod.is_sequencer_only():
                    continue
                if prod.engine == inst.engine:
                    move.append(dn)
            if move:
                sync_set = inst.sync_dependency_set_copy()
                nosync_set = inst.nosync_dependency_set_copy()
                for dn in move:
                    sync_set.discard(dn)
                    nosync_set.add(dn)
                inst.set_sync_dependencies(sync_set)
                inst.set_nosync_dependencies(nosync_set)

    def tensor_tensor(eng, out, in0, in1):
        """Plain 2-tensor elementwise add on DVE/Pool (InstTensorTensor gets
        the 2x 16-bit DVE mode; scalar_tensor_tensor does not)."""
        return eng.add_instruction(
            mybir.InstTensorTensor(
                name=nc.get_next_instruction_name(),
                op=mybir.AluOpType.add,
                ins=[eng.lower_ap(in0), eng.lower_ap(in1)],
                outs=[eng.lower_ap(out)],
            )
        )

    def tensor_copy(eng, out, in_):
        """Elementwise copy (with dtype cast) on DVE/Pool."""
        return eng.add_instruction(
            mybir.InstTensorCopy(
                name=nc.get_next_instruction_name(),
                ins=[eng.lower_ap(in_)],
                outs=[eng.lower_ap(out)],
            )
        )

    nc = bass.Bass(
        "TRN2", target_bir_lowering=False, detect_race_conditions=not relax
    )
    x_dram = nc.dram_tensor(
        "x", [n_strips, 128, STRIP_W], F16, kind="ExternalInput"
    )
    w_dram = nc.dram_tensor("w", [128, 128], F16, kind="ExternalInput")
    y_dram = nc.dram_tensor(
        "y", [n_strips, 128, 64 * S], F16, kind="ExternalOutput"
    )

    with tile.TileContext(nc) as tc:
        with (
            tc.tile_pool(name="pers", bufs=1) as pers,
            tc.tile_pool(name="psum", bufs=2, space="PSUM") as pp,
        ):
            wt = pers.tile([128, 128], F16, tag="wt")
            nc.sync.dma_start(wt[:], w_dram[:])

            x_tiles = [
                pers.tile([128, STRIP_W], F16, tag=f"xs{i}", name=f"xst{i}")
                for i in range(n_strips)
            ]
            v0_bufs = [
                pers.tile([128, V0_W], F16, tag=f"v0_{i}", name=f"v0b{i}")
                for i in range(2)
            ]
            v1_bufs = [
                pers.tile([128, V0_W], F16, tag=f"v1_{i}", name=f"v1b{i}")
                for i in range(2)
            ]
            y_bufs = [
                pers.tile([128, 64 * S], F16, tag=f"y{i}", name=f"ybuf{i}")
                for i in range(2)
            ]
            # dedicated poke scratch: pokes only need to make their engine
            # OBSERVE a store-completion semaphore, not touch real buffers
            pk_d = pers.tile([128, 2], F16, tag="pk_d")
            pk_g = pers.tile([128, 2], F16, tag="pk_g")

            # prefetch every strip as two half-loads (two lanes transfer one
            # strip concurrently, halving time-to-first-matmul): no deps ->
            # no waits, SP ring streams them
            HL = STRIP_W // 2
            load_insts = []
            for s in range(n_strips):
                la = nc.sync.dma_start(
                    x_tiles[s][:, 0:HL], x_dram[s][:, 0:HL]
                )
                lb = nc.sync.dma_start(
                    x_tiles[s][:, HL:STRIP_W], x_dram[s][:, HL:STRIP_W]
                )
                load_insts.append((la, lb))

            # absorber: folds the wt-load wait into PE program order so no
            # matmul carries it (they each have their own single WAR wait)
            nc.tensor.ldweights(wt[:])

            store_insts = []
            for s in range(n_strips):
                xb = x_tiles[s]
                v0 = v0_bufs[s % 2]
                v1 = v1_bufs[s % 2]
                yb = y_bufs[s % 2]

                p01 = pp.tile([128, 1024], F32, tag="p01", bufs=2, name=f"p01_{s}")
                p23 = pp.tile([128, 1024], F32, tag="p23", bufs=1, name=f"p23_{s}")
                p4 = pp.tile([128, 512], F32, tag="p4", bufs=2, name=f"p4_{s}")

                def psum_slice(k):
                    w = SLOT * CHUNK_NS[k]
                    if k < 4:
                        t = p01 if k < 2 else p23
                        off = 512 * (k % 2)
                        return t[:, off : off + w]
                    return p4[:, 0:w]

                # per-strip absorbers: fold the two xb-half-load waits into
                # PE program order so each chunk's first matmul carries only
                # its single PSUM-WAR wait (walrus allows one sem wait per
                # matmul)
                ldwa = nc.tensor.ldweights(wt[:])
                add_dep_helper(ldwa, load_insts[s][0], sync=True, reason="x lo")
                ldwb = nc.tensor.ldweights(wt[:])
                add_dep_helper(ldwb, load_insts[s][1], sync=True, reason="x hi")

                # ---- PE: w = (V.x) (*)_h [1,1], chunk 4 first ----
                # chunk k covers w-positions [2+66*t0, +66*ns); tap e reads
                # xb cols shifted by e.
                for k in (4, 0, 1, 2, 3):
                    t0, ns = CHUNK_T0[k], CHUNK_NS[k]
                    base = 2 + SLOT * t0
                    wk = SLOT * ns
                    dst = psum_slice(k)
                    for e in (0, 1):
                        nc.tensor.matmul(
                            dst,
                            wt[:],
                            xb[:, base + e : base + e + wk],
                            start=(e == 0),
                            stop=(e == 1),
                        )

                # ---- ACT: v0 = fp16(w), order (4, 01, 23) ----
                # poke folds the v0-buffer WAR (DVE v1-add of strip s-2)
                # into ACT program order. (GPSIMD cannot read PSUM on TRN2,
                # so all PSUM evacuation stays on ACT.)
                nc.scalar.copy(v0[0:1, 0:1], wt[0:1, 0:1])
                nc.scalar.copy(
                    v0[:, SLOT * 28 : V0_W], p4[:, 0 : SLOT * 4]
                )
                nc.scalar.copy(
                    v0[:, 0 : 2 * 462].rearrange("p (a b) -> p a b", b=462),
                    p01[:].rearrange("p (a b) -> p a b", b=512)[:, :, 0:462],
                )
                nc.scalar.copy(
                    v0[:, 2 * 462 : 4 * 462].rearrange("p (a b) -> p a b", b=462),
                    p23[:].rearrange("p (a b) -> p a b", b=512)[:, :, 0:462],
                )

                # ---- DVE: two fp16 [1,1] passes ----
                # poke 1 makes DVE observe store(s-2) completion so the
                # y-add's yb WAR wait elides; poke 2 observes Pool's cp23(s)
                # (which subsumes cp4) so the v1-add carries only the ACT
                # cp01 wait.
                dpk = nc.vector.memset(pk_d[0:1, 0:1], 0.0)
                if s >= 2:
                    add_dep_helper(
                        dpk, store_insts[s - 2], sync=True, reason="yb war"
                    )
                tensor_tensor(
                    nc.vector,
                    v1[:, 0 : V0_W - 1],
                    v0[:, 0 : V0_W - 1],
                    v0[:, 1:V0_W],
                )
                v1s = v1[:].rearrange("p (t u) -> p t u", u=SLOT)
                dst = yb[:].rearrange("p (t w) -> p t w", w=64)
                tensor_tensor(nc.vector, dst, v1s[:, :, 0:64], v1s[:, :, 1:65])

                # ---- store: dense permuted dump (host inverse-permutes),
                # issued from the otherwise-idle GPSIMD engine via SWDGE ----
                # Pool poke observes store(s-2) so store(s)'s lane-order
                # wait elides. The last two strips store in two halves so
                # the kernel tail is not one full-strip DMA on a single
                # lane.
                gpk = nc.gpsimd.memset(pk_g[0:1, 0:1], 0.0)
                if s >= 2:
                    add_dep_helper(
                        gpk, store_insts[s - 2], sync=True, reason="lane order"
                    )
                if s >= n_strips - 2:
                    nc.gpsimd.dma_start(
                        y_dram[s][:, 0 : 32 * S], yb[:, 0 : 32 * S]
                    )
                    st = nc.gpsimd.dma_start(
                        y_dram[s][:, 32 * S : 64 * S], yb[:, 32 * S : 64 * S]
                    )
                else:
                    st = nc.gpsimd.dma_start(y_dram[s], yb[:])
                store_insts.append(st)

            if relax:
                relax_same_engine_deps(nc)

    if relax:
        _strip_self_satisfied_waits(nc)

    return nc


def _strip_self_satisfied_waits(nc):
    """Post-scheduling: drop sem waits already guaranteed by the issuing
    engine's own instruction stream (e.g. PE waiting on the PE semaphore for
    a PSUM-slot WAW against its own earlier matmuls — the pool allocator
    emits these during scheduling, after the dep-relaxation pass ran).

    Safe because an engine's compute instructions complete in stream order,
    and only increments issued synchronously by THIS engine's earlier
    non-DMA instructions are counted (DMA completions are asynchronous and
    excluded). Walrus allows one sem wait per instruction, so these
    redundant self-waits are the difference between compiling and not.
    """
    from concourse.tile_scheduler import DMAInst

    cum: dict = {}
    for inst in nc.all_instructions():
        si = inst.sync_info
        if si is None:
            continue
        c = cum.setdefault(str(inst.engine), {})
        waits = list(si.on_wait)
        keep = [
            w
            for w in waits
            if not (
                w.sync_type == "semaphore"
                and w.wait_mode == "sem-ge-imm"
                and w.wait_reg is None
                and c.get(w.ant_name, 0) >= w.wait_value
            )
        ]
        if len(keep) != len(waits):
            si.on_wait = keep
        if not isinstance(inst, DMAInst):
            for u in si.on_update:
                if u.sync_type == "semaphore" and u.update_mode == "sem-inc":
                    c[u.ant_name] = c.get(u.ant_name, 0) + (u.update_value or 1)


def build_weights(kern: np.ndarray) -> np.ndarray:
    """Vertical banded lhsT [K=128(in row), M=128(out row)], block-diag per
    image: V[64j + r', 64j + r] = Vw[r+1-r'] with Vw = kern[:, 0]; the
    horizontal [1,3,3,1] factor is applied by the [1,1] tap pair + two DVE
    add passes."""
    kern = np.asarray(kern, np.float32)
    Vw = kern[:, 0]
    h = kern[0, :] / kern[0, 0]
    assert np.allclose(h, [1.0, 3.0, 3.0, 1.0], atol=1e-5), h
    assert np.allclose(kern, np.outer(Vw, h), atol=1e-7)
    v = np.zeros((128, 128), np.float32)
    for blk in (0, 64):
        for r in range(64):
            for a in range(4):
                rp = r + 1 - a
                if 0 <= rp < 64:
                    v[blk + rp, blk + r] = Vw[a]
    return v.astype(np.float16)


def marshal(x: np.ndarray, n_cores: int = N_CORES) -> np.ndarray:
    """Full (G, 64, 64) f32 -> prepadded per-core fp16 strips
    [n_cores, N_STRIPS, 128, STRIP_W]."""
    G = x.shape[0]
    n_strips = G // (n_cores * 2 * S)
    xr = x.reshape(n_cores, n_strips, S, 2, H, W)          # [c, s, t, j, r, w]
    out = np.zeros((n_cores, n_strips, 128, STRIP_W), np.float16)
    view = out[:, :, :, LEAD : LEAD + SLOT * S].reshape(
        n_cores, n_strips, 2, H, S, SLOT
    )                                                       # [c, s, j, r, t, u]
    view[..., 0:64] = xr.transpose(0, 1, 3, 4, 2, 5)
    return out


def unmarshal_y(yp: np.ndarray) -> np.ndarray:
    """Per-core permuted output [n_cores, N_STRIPS, 128, 64*S] fp16 ->
    (G, 64, 64) f32."""
    n_cores, n_strips = yp.shape[0], yp.shape[1]
    v = yp.reshape(n_cores, n_strips, 2, H, S, 64)         # [c, s, j, r, t, w]
    return np.ascontiguousarray(
        v.transpose(0, 1, 4, 2, 3, 5)                      # [c, s, t, j, r, w]
    ).astype(np.float32).reshape(n_cores * n_strips * 2 * S, H, W)


def make_in_maps(x: np.ndarray, kern: np.ndarray):
    """x: (B, C, 64, 64) f32 -> per-core input maps."""
    G = x.shape[0] * x.shape[1]
    xp = marshal(np.asarray(x, np.float32).reshape(G, H, W))
    w_all = build_weights(kern)
    return [{"x": xp[c], "w": w_all} for c in range(N_CORES)]


_CACHE: dict = {}


def _get_nc():
    if "nc" not in _CACHE:
        _CACHE["nc"] = build_nc(n_strips=N_STRIPS)
    return _CACHE["nc"]


def kernel(x, kernel):
    x = np.ascontiguousarray(np.asarray(x, dtype=np.float32))
    kern = np.asarray(kernel, dtype=np.float32)
    B, C, HH, WW = x.shape

    nc = _get_nc()
    in_maps = make_in_maps(x, kern)
    res = run_bass_kernel_spmd(nc, in_maps, list(range(N_CORES)))
    yp = np.stack([res.results[c]["y"] for c in range(N_CORES)], axis=0)
    return unmarshal_y(yp).reshape(B, C, HH, WW).astype(np.float32)


if __name__ == "__main__":
    # quick self-check against numpy on random data (runs on hardware)
    rng = np.random.default_rng(0)
    x = rng.standard_normal((16, 512, 64, 64), dtype=np.float32)
    k1 = np.array([1.0, 3.0, 3.0, 1.0], np.float32)
    kern = np.outer(k1, k1)
    kern /= kern.sum()
    y = kernel(x, kern)
    print("out shape", y.shape, "dtype", y.dtype)
